# revision 1
# baseline (speedup 1.0000x reference)
"""Multi-head causal attention (B=4, T=2048, C=1024, H=16) on 8 TRN2 cores.

Sharding: core c handles batch b = c//2 and heads hg*8..hg*8+8 (hg = c%2).
Each core computes its 8 heads' attention probabilities (full [8, T, T] f32,
lower triangle only -- output buffers are pre-zeroed by the runtime) and a
partial output projection y_part [T, C]; the host sums the two partials per
batch.

Returns (out, attn) matching the reference.
"""

import math

import numpy as np
import ml_dtypes

import concourse.bass as bass
import concourse.mybir as mybir
import concourse.tile as tile
from concourse import bacc
from concourse.bass_utils import run_bass_kernel_spmd

BF16 = ml_dtypes.bfloat16

B = 4
C = 1024
N_HEAD = 16
HEAD_DIM = 64
HPC = 8          # heads per core
DLOC = HPC * HEAD_DIM  # 512
NEG = -1.0e30
SCALE = 1.0 / math.sqrt(HEAD_DIM)

_NC_CACHE = {}


def build_nc(T):
    nc = bacc.Bacc("TRN2", target_bir_lowering=False, debug=False)
    dt = mybir.dt
    f32 = dt.float32
    bf = dt.bfloat16
    NB = T // 128          # q/k blocks of 128
    NCC = C // 128         # contraction chunks for projections

    xT_d = nc.dram_tensor("xT", [C, T], bf, kind="ExternalInput")
    wq_d = nc.dram_tensor("wqT", [C, DLOC], bf, kind="ExternalInput")
    wk_d = nc.dram_tensor("wkT", [C, DLOC], bf, kind="ExternalInput")
    wv_d = nc.dram_tensor("wvT", [C, DLOC], bf, kind="ExternalInput")
    wo_d = nc.dram_tensor("woT", [DLOC, C], bf, kind="ExternalInput")
    nm1_d = nc.dram_tensor("nm1", [128, 128], f32, kind="ExternalInput")
    nm2_d = nc.dram_tensor("nm2", [128, 128], f32, kind="ExternalInput")

    attn_d = nc.dram_tensor("attn", [HPC, T, T], f32, kind="ExternalOutput")
    y_d = nc.dram_tensor("y", [T, C], f32, kind="ExternalOutput")

    Exp = mybir.ActivationFunctionType.Exp
    mult = mybir.AluOpType.mult

    with tile.TileContext(nc) as tc:
        with (
            tc.tile_pool(name="persist", bufs=1) as per,
            tc.tile_pool(name="scratch_dram", bufs=2, space="DRAM") as drp,
        ):
            nm1 = per.tile([128, 128], f32, tag="nm1")
            nm2 = per.tile([128, 128], f32, tag="nm2")
            nc.sync.dma_start(out=nm1, in_=nm1_d[:, :])
            nc.sync.dma_start(out=nm2, in_=nm2_d[:, :])
            wo_sb = per.tile([128, DLOC // 128, C], bf, tag="wo")
            nc.sync.dma_start(
                out=wo_sb, in_=wo_d.ap().rearrange("(j p) c -> p j c", p=128)
            )
            # persistent activations
            qT = per.tile([128, DLOC // 128, T], bf, tag="qT")
            kT = per.tile([128, DLOC // 128, T], bf, tag="kT")
            v_sb = per.tile([128, NB, HPC * 65], bf, tag="v")
            outT = per.tile([128, DLOC // 128, T], bf, tag="outT")
            nc.vector.memset(v_sb, 1.0)  # ones columns survive at 65*h+64

            # ---------------- QKV projections ----------------
            with (
                tc.tile_pool(name="qkv_w", bufs=1) as wp,
                tc.tile_pool(name="qkv_ps", bufs=4, space="PSUM") as qps,
            ):
                xT_sb = wp.tile([128, NCC, T], bf, tag="xT")
                nc.sync.dma_start(
                    out=xT_sb, in_=xT_d.ap().rearrange("(cc p) t -> p cc t", p=128)
                )
                w_tiles = {}
                for name, d in (("wq", wq_d), ("wk", wk_d), ("wv", wv_d)):
                    w_sb = wp.tile([128, NCC, DLOC], bf, tag=name)
                    nc.sync.dma_start(
                        out=w_sb, in_=d.ap().rearrange("(cc p) o -> p cc o", p=128)
                    )
                    w_tiles[name] = w_sb

                # q, k -> [dout partition, t free] (transposed layout)
                for name, dst in (("wq", qT), ("wk", kT)):
                    w_sb = w_tiles[name]
                    for j in range(DLOC // 128):
                        for tci in range(T // 512):
                            ps = qps.tile([128, 512], f32, tag="qkps")
                            for cc in range(NCC):
                                nc.tensor.matmul(
                                    ps,
                                    w_sb[:, cc, j * 128:(j + 1) * 128],
                                    xT_sb[:, cc, tci * 512:(tci + 1) * 512],
                                    start=(cc == 0),
                                    stop=(cc == NCC - 1),
                                )
                            nc.vector.tensor_copy(
                                dst[:, j, tci * 512:(tci + 1) * 512], ps
                            )
                # v -> [t partition, dout free], interleaved with ones cols
                w_sb = w_tiles["wv"]
                for tt in range(NB):
                    ps = qps.tile([128, 512], f32, tag="qkps")
                    for cc in range(NCC):
                        nc.tensor.matmul(
                            ps,
                            xT_sb[:, cc, tt * 128:(tt + 1) * 128],
                            w_sb[:, cc, :],
                            start=(cc == 0),
                            stop=(cc == NCC - 1),
                        )
                    nc.vector.tensor_copy(
                        v_sb[:, tt, :].rearrange("p (h e) -> p h e", e=65)[:, :, 0:64],
                        ps.rearrange("p (h e) -> p h e", e=64),
                    )

            # ---------------- attention ----------------
            with (
                tc.tile_pool(name="attn_sb", bufs=1) as sbp,
                tc.tile_pool(name="sps_ps", bufs=2, space="PSUM") as pp,
                tc.tile_pool(name="av_ps", bufs=1, space="PSUM") as pav,
            ):
                for h in range(HPC):
                    jh = h // 2
                    p0 = (h % 2) * 64

                    # ---- pass 1: scores [q, k]; exp; normalize; attn out ----
                    for qb in range(NB):
                        W = (qb + 1) * 128
                        lhs_q = qT[p0:p0 + 64, jh, qb * 128:(qb + 1) * 128]
                        strips = []
                        sums = []
                        for w0 in range(0, W, 1024):
                            Wc = min(1024, W - w0)
                            sps = pp.tile([128, 1024], f32, tag="sps")
                            for c0 in range(0, Wc, 512):
                                ncol = min(512, Wc - c0)
                                nc.tensor.matmul(
                                    sps[:, c0:c0 + ncol],
                                    lhs_q,
                                    kT[p0:p0 + 64, jh, w0 + c0:w0 + c0 + ncol],
                                    start=True,
                                    stop=True,
                                )
                            if w0 <= W - 128 < w0 + Wc:
                                dc = (W - 128) - w0
                                nc.vector.tensor_add(
                                    sps[:, dc:dc + 128], sps[:, dc:dc + 128], nm1
                                )
                            e1 = sbp.tile([128, 1024], bf, tag="e1", bufs=3)
                            ss = sbp.tile([128, 1], f32, tag="ss", bufs=4)
                            nc.scalar.activation(
                                e1[:, :Wc], sps[:, :Wc], Exp,
                                scale=SCALE, accum_out=ss,
                            )
                            strips.append((e1, w0, Wc))
                            sums.append(ss)
                        if len(sums) == 2:
                            st = sbp.tile([128, 1], f32, tag="st", bufs=2)
                            nc.vector.tensor_add(st, sums[0], sums[1])
                        else:
                            st = sums[0]
                        rec = sbp.tile([128, 1], f32, tag="rec", bufs=2)
                        nc.vector.reciprocal(rec, st)
                        for e1, w0, Wc in strips:
                            a1 = sbp.tile([128, 1024], bf, tag="a1", bufs=3)
                            nc.vector.tensor_scalar_mul(a1[:, :Wc], e1[:, :Wc], rec)
                            nc.gpsimd.dma_start(
                                out=attn_d[h, qb * 128:(qb + 1) * 128, w0:w0 + Wc],
                                in_=a1[:, :Wc],
                            )

                    # ---- pass 2: scores [k, q]; exp; av accumulate ----
                    av = pav.tile([65, T], f32, tag="av")
                    for kb in range(NB):
                        q0 = kb * 128
                        lhs_k = kT[p0:p0 + 64, jh, kb * 128:(kb + 1) * 128]
                        vh = v_sb[:, kb, h * 65:(h + 1) * 65]
                        for w0 in range(q0, T, 1024):
                            Wc = min(1024, T - w0)
                            sps = pp.tile([128, 1024], f32, tag="sps")
                            for c0 in range(0, Wc, 512):
                                ncol = min(512, Wc - c0)
                                nc.tensor.matmul(
                                    sps[:, c0:c0 + ncol],
                                    lhs_k,
                                    qT[p0:p0 + 64, jh, w0 + c0:w0 + c0 + ncol],
                                    start=True,
                                    stop=True,
                                )
                            if w0 == q0:
                                nc.vector.tensor_add(
                                    sps[:, 0:128], sps[:, 0:128], nm2
                                )
                            e2 = sbp.tile([128, 1024], bf, tag="e2", bufs=3)
                            nc.scalar.activation(
                                e2[:, :Wc], sps[:, :Wc], Exp, scale=SCALE
                            )
                            for c0 in range(0, Wc, 512):
                                ncol = min(512, Wc - c0)
                                nc.tensor.matmul(
                                    av[:, w0 + c0:w0 + c0 + ncol],
                                    vh,
                                    e2[:, c0:c0 + ncol],
                                    start=(kb == 0),
                                    stop=(kb == NB - 1),
                                    skip_group_check=True,
                                )
                    # normalize av rows by the ones-column sums (row 64)
                    rrow = sbp.tile([1, T], f32, tag="rrow", bufs=2)
                    nc.vector.reciprocal(rrow, av[64:65, :])
                    rd = drp.tile([1, T], f32, tag="rd")
                    nc.sync.dma_start(out=rd, in_=rrow)
                    rb = sbp.tile([64, T], f32, tag="rb", bufs=2)
                    nc.gpsimd.dma_start(out=rb, in_=rd.to_broadcast([64, T]))
                    nc.vector.tensor_tensor(
                        out=outT[p0:p0 + 64, jh, :],
                        in0=av[0:64, :],
                        in1=rb,
                        op=mult,
                    )

            # ---------------- output projection ----------------
            with (
                tc.tile_pool(name="proj_sb", bufs=3) as ysb,
                tc.tile_pool(name="proj_ps", bufs=2, space="PSUM") as yps,
            ):
                for tt in range(NB):
                    for n2 in range(C // 512):
                        ps = yps.tile([128, 512], f32, tag="yps")
                        for j in range(DLOC // 128):
                            nc.tensor.matmul(
                                ps,
                                outT[:, j, tt * 128:(tt + 1) * 128],
                                wo_sb[:, j, n2 * 512:(n2 + 1) * 512],
                                start=(j == 0),
                                stop=(j == DLOC // 128 - 1),
                            )
                        y_sb = ysb.tile([128, 512], f32, tag="ysb")
                        nc.vector.tensor_copy(y_sb, ps)
                        nc.sync.dma_start(
                            out=y_d[tt * 128:(tt + 1) * 128, n2 * 512:(n2 + 1) * 512],
                            in_=y_sb,
                        )

    nc.compile()
    return nc


def get_nc(T):
    if T not in _NC_CACHE:
        _NC_CACHE[T] = build_nc(T)
    return _NC_CACHE[T]


def make_masks():
    r = np.arange(128)[:, None]
    c = np.arange(128)[None, :]
    nm1 = np.where(c > r, NEG, 0.0).astype(np.float32)   # [q,k] local: kill k > q
    nm2 = np.where(c < r, NEG, 0.0).astype(np.float32)   # [k,q] local: kill q < k
    return nm1, nm2


def make_in_maps(x, Wq, Wk, Wv, Wo, n_cores=8):
    nm1, nm2 = make_masks()
    in_maps = []
    for core in range(n_cores):
        b = core // 2
        hg = core % 2
        rows = slice(hg * DLOC, (hg + 1) * DLOC)
        in_maps.append({
            "xT": np.ascontiguousarray(x[b].T).astype(BF16),
            "wqT": np.ascontiguousarray(Wq[rows, :].T).astype(BF16),
            "wkT": np.ascontiguousarray(Wk[rows, :].T).astype(BF16),
            "wvT": np.ascontiguousarray(Wv[rows, :].T).astype(BF16),
            "woT": np.ascontiguousarray(Wo[:, rows].T).astype(BF16),
            "nm1": nm1,
            "nm2": nm2,
        })
    return in_maps


def kernel(x, Wq, Wk, Wv, Wo, _trace=False):
    x = np.asarray(x)
    T = x.shape[1]
    nc = get_nc(T)
    in_maps = make_in_maps(x, Wq, Wk, Wv, Wo)
    res = run_bass_kernel_spmd(nc, in_maps, list(range(8)), trace=_trace)
    out = np.empty((B, T, C), np.float32)
    attn = np.empty((B, N_HEAD, T, T), np.float32)
    for core in range(8):
        b = core // 2
        hg = core % 2
        attn[b, hg * HPC:(hg + 1) * HPC] = res.results[core]["attn"]
    for b in range(B):
        out[b] = res.results[2 * b]["y"] + res.results[2 * b + 1]["y"]
    if _trace:
        return (out, attn), res
    return (out, attn)


# revision 9
# speedup vs baseline: 73.7186x; 73.7186x over previous
"""Multi-head causal attention (B=4, T=2048, C=1024, H=16) on 8 TRN2 cores.

Sharding: core c handles batch b = c//2 and heads hg*8..hg*8+8 (hg = c%2).
Each core computes its 8 heads' attention probabilities (full [8, T, T] f32,
lower triangle only -- output buffers are pre-zeroed by the runtime) and a
partial output projection y_part [T, C]; the host sums the two partials per
batch.

Returns (out, attn) matching the reference.
"""

import math

import numpy as np
import ml_dtypes

import concourse.bass as bass
import concourse.mybir as mybir
import concourse.tile as tile
from concourse import bacc
from concourse.bass_utils import run_bass_kernel_spmd

BF16 = ml_dtypes.bfloat16

B = 4
C = 1024
N_HEAD = 16
HEAD_DIM = 64
HPC = 8          # heads per core
DLOC = HPC * HEAD_DIM  # 512
NEG = -1.0e30
SCALE = 1.0 / math.sqrt(HEAD_DIM)

_NC_CACHE = {}


def build_nc(T, reps=1):
    nc = bacc.Bacc("TRN2", target_bir_lowering=False, debug=False)
    dt = mybir.dt
    f32 = dt.float32
    bf = dt.bfloat16
    NB = T // 128          # q/k blocks of 128
    NCC = C // 128         # contraction chunks for projections

    xT_d = nc.dram_tensor("xT", [C, T], bf, kind="ExternalInput")
    wq_d = nc.dram_tensor("wqT", [C, DLOC], bf, kind="ExternalInput")
    wk_d = nc.dram_tensor("wkT", [C, DLOC], bf, kind="ExternalInput")
    wv_d = nc.dram_tensor("wvT", [C, DLOC], bf, kind="ExternalInput")
    wo_d = nc.dram_tensor("woT", [DLOC, C], bf, kind="ExternalInput")
    nm1_d = nc.dram_tensor("nm1", [128, 128], bf, kind="ExternalInput")
    nm2_d = nc.dram_tensor("nm2", [128, 128], bf, kind="ExternalInput")
    id_d = nc.dram_tensor("ident", [128, 128], bf, kind="ExternalInput")

    attn_d = nc.dram_tensor("attn", [HPC, T, T], f32, kind="ExternalOutput")
    y_d = nc.dram_tensor("y", [T, C], f32, kind="ExternalOutput")

    Exp = mybir.ActivationFunctionType.Exp
    mult = mybir.AluOpType.mult

    with tile.TileContext(nc) as tc:
        with (
            tc.tile_pool(name="persist", bufs=1) as per,
            tc.tile_pool(name="scratch_dram", bufs=2, space="DRAM") as drp,
        ):
            nm1 = per.tile([128, 128], bf, tag="nm1")
            nm2 = per.tile([128, 128], bf, tag="nm2")
            nc.sync.dma_start(out=nm1, in_=nm1_d[:, :])
            nc.sync.dma_start(out=nm2, in_=nm2_d[:, :])
            ident = per.tile([128, 128], bf, tag="ident")
            nc.sync.dma_start(out=ident, in_=id_d[:, :])
            wo_sb = per.tile([128, DLOC // 128, C], bf, tag="wo")
            nc.sync.dma_start(
                out=wo_sb, in_=wo_d.ap().rearrange("(j p) c -> p j c", p=128)
            )
            # persistent activations
            qT = per.tile([128, DLOC // 128, T], bf, tag="qT")
            kT = per.tile([128, DLOC // 128, T], bf, tag="kT")
            v_sb = per.tile([128, NB, DLOC], bf, tag="v")
            outT = per.tile([128, DLOC // 128, T], bf, tag="outT")
            out_all = per.tile([128, NB, DLOC], bf, tag="out_all")

            for _rep in range(reps):
                build_body(nc, tc, per, drp, locals())

    nc.compile()
    return nc


def _emit_av(nc, pav, item, stages, v_sb, out_all, hp, f32):
    qb, recs = item
    for hh in range(2):
        h = 2 * hp + hh
        av = pav.tile([128, 64], f32, name=f"av{hh}_{hp}_{qb}",
                      tag=f"av{hh}", bufs=1)
        for kb in range(qb + 1):
            nc.tensor.matmul(
                av,
                stages[hh][:, kb, (qb % 4) * 128:(qb % 4 + 1) * 128],
                v_sb[:, kb, h * 64:(h + 1) * 64],
                start=(kb == 0),
                stop=(kb == qb),
            )
        nc.vector.tensor_scalar_mul(
            out_all[:, qb, h * 64:(h + 1) * 64], av, recs[hh]
        )


def build_body(nc, tc, per, drp, env):
    dt = mybir.dt
    f32 = dt.float32
    bf = dt.bfloat16
    T = env["T"]
    NB = env["NB"]
    NCC = env["NCC"]
    qT = env["qT"]; kT = env["kT"]; v_sb = env["v_sb"]; outT = env["outT"]
    out_all = env["out_all"]; ident = env["ident"]
    nm1 = env["nm1"]; nm2 = env["nm2"]; wo_sb = env["wo_sb"]
    xT_d = env["xT_d"]; wq_d = env["wq_d"]; wk_d = env["wk_d"]; wv_d = env["wv_d"]
    attn_d = env["attn_d"]; y_d = env["y_d"]
    Exp = env["Exp"]; mult = env["mult"]
    if True:
            # ---------------- QKV projections ----------------
            with (
                tc.tile_pool(name="qkv_w", bufs=1) as wp,
                tc.tile_pool(name="qkv_ps", bufs=4, space="PSUM") as qps,
            ):
                xT_sb = wp.tile([128, NCC, T], bf, tag="xT")
                nc.sync.dma_start(
                    out=xT_sb, in_=xT_d.ap().rearrange("(cc p) t -> p cc t", p=128)
                )
                w_tiles = {}
                for name, d in (("wq", wq_d), ("wk", wk_d), ("wv", wv_d)):
                    w_sb = wp.tile([128, NCC, DLOC], bf, tag=name)
                    nc.sync.dma_start(
                        out=w_sb, in_=d.ap().rearrange("(cc p) o -> p cc o", p=128)
                    )
                    w_tiles[name] = w_sb

                # q, k -> [dout partition, t free] (transposed layout)
                for name, dst in (("wq", qT), ("wk", kT)):
                    w_sb = w_tiles[name]
                    for j in range(DLOC // 128):
                        for tci in range(T // 512):
                            ps = qps.tile([128, 512], f32, tag="qkps")
                            for cc in range(NCC):
                                nc.tensor.matmul(
                                    ps,
                                    w_sb[:, cc, j * 128:(j + 1) * 128],
                                    xT_sb[:, cc, tci * 512:(tci + 1) * 512],
                                    start=(cc == 0),
                                    stop=(cc == NCC - 1),
                                )
                            nc.vector.tensor_copy(
                                dst[:, j, tci * 512:(tci + 1) * 512], ps
                            )
                # v -> [t partition, dout free], interleaved with ones cols
                w_sb = w_tiles["wv"]
                for tt in range(NB):
                    ps = qps.tile([128, 512], f32, tag="qkps")
                    for cc in range(NCC):
                        nc.tensor.matmul(
                            ps,
                            xT_sb[:, cc, tt * 128:(tt + 1) * 128],
                            w_sb[:, cc, :],
                            start=(cc == 0),
                            stop=(cc == NCC - 1),
                        )
                    nc.vector.tensor_copy(v_sb[:, tt, :], ps)

            # ---------------- attention ----------------
            # Head pairs (partitions 0-63 / 64-127 of one dblock). Causal mask
            # applied on the PE via an accumulated identity @ nm1 matmul.
            # av runs in [q, d] orientation (lhsT = staged attn_T), so the
            # pass-1 per-partition reciprocals normalize it directly.
            with (
                tc.tile_pool(name="attn_sb", bufs=1) as sbp,
                tc.tile_pool(name="sps_ps", bufs=1, space="PSUM") as pp,
                tc.tile_pool(name="av_ps", bufs=1, space="PSUM") as pav,
            ):
                for hp in range(HPC // 2):
                    jh = hp
                    stages = [
                        sbp.tile([128, NB, 512], bf, name=f"stage{i}_{hp}",
                                 tag=f"stage{i}", bufs=1)
                        for i in range(2)
                    ]
                    pend = []
                    for qc in range(NB // 4):
                        for qb in range(4 * qc, 4 * qc + 4):
                            W = (qb + 1) * 128
                            e1s = []
                            sums = [[], []]
                            for hh in range(2):
                                e1 = sbp.tile([128, 2048], bf,
                                              name=f"e1_{hh}_{hp}_{qb}",
                                              tag=f"e1_{hh}", bufs=2)
                                e1s.append(e1)
                            for c0 in range(0, W, 512):
                                ncol = min(512, W - c0)
                                for hh in range(2):
                                    p0 = hh * 64
                                    lhs_q = qT[p0:p0 + 64, jh,
                                               qb * 128:(qb + 1) * 128]
                                    sps = pp.tile([128, 512], f32,
                                                  name=f"sps{hh}_{hp}_{qb}_{c0}",
                                                  tag=f"sps{hh}", bufs=3)
                                    nc.tensor.matmul(
                                        sps[:, :ncol],
                                        lhs_q,
                                        kT[p0:p0 + 64, jh, c0:c0 + ncol],
                                        start=True,
                                        stop=(c0 + 512 < W),
                                    )
                                    if c0 + 512 >= W:
                                        # diagonal block: add causal mask on PE
                                        dc = (W - 128) - c0
                                        nc.tensor.matmul(
                                            sps[:, dc:dc + 128],
                                            ident,
                                            nm1,
                                            start=False,
                                            stop=True,
                                            skip_group_check=True,
                                        )
                                    ss = sbp.tile([128, 1], f32,
                                                  name=f"ss{hh}_{hp}_{qb}_{c0}",
                                                  tag=f"ss{hh}", bufs=8)
                                    nc.scalar.activation(
                                        e1s[hh][:, c0:c0 + ncol], sps[:, :ncol],
                                        Exp, scale=SCALE, accum_out=ss,
                                    )
                                    sums[hh].append(ss)
                            recs = []
                            for hh in range(2):
                                st = sums[hh][0]
                                for ss in sums[hh][1:]:
                                    st2 = sbp.tile([128, 1], f32,
                                                   name=f"st{hh}_{hp}_{qb}_{id(ss)}",
                                                   tag=f"st{hh}", bufs=4)
                                    nc.vector.tensor_add(st2, st, ss)
                                    st = st2
                                rec = sbp.tile([128, 1], f32,
                                               name=f"rec{hh}_{hp}_{qb}",
                                               tag=f"rec{hh}", bufs=4)
                                nc.vector.reciprocal(rec, st)
                                recs.append(rec)
                            for hh in range(2):
                                e1 = e1s[hh]
                                a1 = sbp.tile([128, 2048], bf,
                                              name=f"a1_{hh}_{hp}_{qb}",
                                              tag=f"a1_{hh}", bufs=2)
                                nc.vector.tensor_scalar_mul(
                                    a1[:, :W], e1[:, :W], recs[hh]
                                )
                                nc.gpsimd.dma_start(
                                    out=attn_d[2 * hp + hh,
                                               qb * 128:(qb + 1) * 128, 0:W],
                                    in_=a1[:, :W],
                                )
                                eng = nc.sync if hh == 0 else nc.scalar
                                eng.dma_start_transpose(
                                    stages[hh][:, 0:W // 128,
                                               (qb % 4) * 128:(qb % 4 + 1) * 128],
                                    e1[:, :W],
                                )
                            # defer av one qb so transpose latency overlaps
                            pend.append((qb, recs))
                            if len(pend) > 1:
                                _emit_av(nc, pav, pend.pop(0), stages, v_sb,
                                         out_all, hp, f32)
                    while pend:
                        _emit_av(nc, pav, pend.pop(0), stages, v_sb,
                                 out_all, hp, f32)

                # assemble outT via 16 batched transposes
                for tt in range(NB):
                    eng = nc.sync if tt % 2 == 0 else nc.scalar
                    eng.dma_start_transpose(
                        outT[:, :, tt * 128:(tt + 1) * 128],
                        out_all[:, tt, :],
                    )

            # ---------------- output projection ----------------
            with (
                tc.tile_pool(name="proj_sb", bufs=3) as ysb,
                tc.tile_pool(name="proj_ps", bufs=2, space="PSUM") as yps,
            ):  # noqa

                for tt in range(NB):
                    for n2 in range(C // 512):
                        ps = yps.tile([128, 512], f32, tag="yps")
                        for j in range(DLOC // 128):
                            nc.tensor.matmul(
                                ps,
                                outT[:, j, tt * 128:(tt + 1) * 128],
                                wo_sb[:, j, n2 * 512:(n2 + 1) * 512],
                                start=(j == 0),
                                stop=(j == DLOC // 128 - 1),
                            )
                        y_sb = ysb.tile([128, 512], f32, tag="ysb")
                        nc.vector.tensor_copy(y_sb, ps)
                        nc.sync.dma_start(
                            out=y_d[tt * 128:(tt + 1) * 128, n2 * 512:(n2 + 1) * 512],
                            in_=y_sb,
                        )


def get_nc(T, reps=1):
    key = (T, reps)
    if key not in _NC_CACHE:
        _NC_CACHE[key] = build_nc(T, reps)
    return _NC_CACHE[key]


def make_masks():
    r = np.arange(128)[:, None]
    c = np.arange(128)[None, :]
    nm1 = np.where(c > r, NEG, 0.0).astype(BF16)   # [q,k] local: kill k > q
    nm2 = np.where(c < r, NEG, 0.0).astype(BF16)   # [k,q] local: kill q < k
    return nm1, nm2


def make_in_maps(x, Wq, Wk, Wv, Wo, n_cores=8):
    nm1, nm2 = make_masks()
    in_maps = []
    for core in range(n_cores):
        b = core // 2
        hg = core % 2
        rows = slice(hg * DLOC, (hg + 1) * DLOC)
        in_maps.append({
            "xT": np.ascontiguousarray(x[b].T).astype(BF16),
            "wqT": np.ascontiguousarray(Wq[rows, :].T).astype(BF16),
            "wkT": np.ascontiguousarray(Wk[rows, :].T).astype(BF16),
            "wvT": np.ascontiguousarray(Wv[rows, :].T).astype(BF16),
            "woT": np.ascontiguousarray(Wo[:, rows].T).astype(BF16),
            "nm1": nm1,
            "nm2": nm2,
            "ident": np.eye(128, dtype=BF16),
        })
    return in_maps


def kernel(x, Wq, Wk, Wv, Wo, _trace=False):
    x = np.asarray(x)
    T = x.shape[1]
    nc = get_nc(T)
    in_maps = make_in_maps(x, Wq, Wk, Wv, Wo)
    res = run_bass_kernel_spmd(nc, in_maps, list(range(8)), trace=_trace)
    out = np.empty((B, T, C), np.float32)
    attn = np.empty((B, N_HEAD, T, T), np.float32)
    for core in range(8):
        b = core // 2
        hg = core % 2
        attn[b, hg * HPC:(hg + 1) * HPC] = res.results[core]["attn"]
    for b in range(B):
        out[b] = res.results[2 * b]["y"] + res.results[2 * b + 1]["y"]
    if _trace:
        return (out, attn), res
    return (out, attn)
